# revision 6
# baseline (speedup 1.0000x reference)
"""Trainium2 8-core Bass kernel for nn_AntisymmetricExpGenerator.

Reference computation (H=2048, B=512):
    A      = 0.5*(W - W.T)                      (antisymmetric)
    rec    = h @ expm(A*d).T
    b      = cat([du, u]) @ Bw.T
    M      = inv(A) @ (expm(A*d) - I)
    y      = (rec + b @ M.T) @ Cw.T

Key identity: inv(A) @ (expm(A*d) - I) = d * phi1(A*d) where
phi1(z) = (e^z - 1)/z = sum_k z^k/(k+1)!  is ENTIRE - no inverse and no
dense (H,H) expm/inv is needed.  With ||A*d|| ~ 8e-3 the series
converges after 2-3 terms (truncation ~1e-5 relative, far below fp32
matmul noise):

    b @ M.T = d*(b + (d/4)*b@Abar.T + O(1e-5))        Abar = W - W.T
    rec     = h + (d/2)*h@Abar.T + O(3e-5)

so everything reduces to skinny matmuls of the batch block against
Abar - never a 2048^3 product.

Distribution (8 cores): H dim sharded 256 rows/core.  Activations live
transposed (feature on partitions, batch on free dim).  Two AllGathers
(the B0|h block, then the H1 block) move (H,513)/(H,512) blocks;
weights are pre-sliced per core on the host (layout only).

dtypes: suppressed paths (scaled by d=0.01) run bf16; the direct path
(H1 -> y through Cw) runs float32r (TF32-like, ~1.5e-4) with fp32 PSUM
accumulation.  End-to-end error vs reference ~2e-4.
"""

import sys

sys.path.insert(0, "/opt/trn_rl_repo")

import numpy as np
import ml_dtypes

import concourse.bass as bass
import concourse.mybir as mybir
import concourse.tile as tile
from concourse import bacc
from concourse.bass_utils import run_bass_kernel_spmd

# problem constants (hardcoded per harness contract)
DELTA = 0.01
B_SZ, U_DIM, DU_DIM, H_DIM, Y_DIM = 512, 1024, 512, 2048, 1024
F_DIM = U_DIM + DU_DIM  # 1536
N_CORES = 8
HS = H_DIM // N_CORES  # 256 rows of H per core
YS = Y_DIM // N_CORES  # 128 rows of y^T per core

F32 = mybir.dt.float32
F32R = mybir.dt.float32r
BF16 = mybir.dt.bfloat16
BF = ml_dtypes.bfloat16

P = 128
NB = B_SZ  # batch free-dim (=512, one PSUM bank of fp32)
KF = F_DIM // P  # 12 k-tiles for stage A
KH = H_DIM // P  # 16 k-tiles for H-contractions
MT = HS // P  # 2 m-tiles per core for H-sharded outputs


def _to_sb_layout(a: np.ndarray, dtype) -> np.ndarray:
    """(K, M) -> (128, (K//128)*M): k-tile kf lands at cols [kf*M,(kf+1)*M).

    Matches an SBUF tile (128, K//128 * M) where slice [:, kf*M:(kf+1)*M]
    is the (128, M) k-tile needed by the tensor engine.
    """
    K, M = a.shape
    assert K % P == 0
    return np.ascontiguousarray(
        a.reshape(K // P, P, M).transpose(1, 0, 2).reshape(P, (K // P) * M)
    ).astype(dtype, copy=False)


def build_nc():
    nc = bacc.Bacc("TRN2", target_bir_lowering=False, debug=False, num_devices=N_CORES)

    # --- per-core DRAM parameters (host-prepared layouts) ---
    # stage A: B0[I_c] = Bw[I_c,:] @ cat^T ;  lhsT = Bw[I_c,:].T  (F, HS)
    catT = nc.dram_tensor("catT", [P, KF * NB], BF16, kind="ExternalInput")
    bwT = nc.dram_tensor("bwT", [P, KF * HS], BF16, kind="ExternalInput")
    # stage S1: Z1[I_c] = W[I_c,:] @ Z0 - W[:,I_c].T @ Z0
    #   lhsT(term1) = (W[I_c,:]).T  (H, HS);  lhsT(term2) = -W[:,I_c]  (H, HS)
    wrowT = nc.dram_tensor("wrowT", [P, KH * HS], BF16, kind="ExternalInput")
    wcolN = nc.dram_tensor("wcolN", [P, KH * HS], BF16, kind="ExternalInput")
    # stage C: yT[J_c] = Cw[J_c,:] @ H1 ; lhsT = Cw[J_c,:].T  (H, YS)
    cwT = nc.dram_tensor("cwT", [P, KH * YS], F32R, kind="ExternalInput")
    # h vector slice for this core, (HS,1) -> stored (128, MT)
    vcol = nc.dram_tensor("vcol", [P, MT], F32, kind="ExternalInput")

    out = nc.dram_tensor("out", [YS, NB], F32, kind="ExternalOutput")

    d = DELTA

    with tile.TileContext(nc) as tc:
        with (
            tc.tile_pool(name="weights", bufs=1) as wpool,
            tc.tile_pool(name="acts", bufs=1) as apool,
            tc.tile_pool(name="psumA", bufs=2, space="PSUM") as psA,
            tc.tile_pool(name="psumM", bufs=2, space="PSUM") as psM,
            tc.tile_pool(name="psumV", bufs=2, space="PSUM") as psV,
            tc.tile_pool(name="psumC", bufs=1, space="PSUM") as psC,
            tc.tile_pool(name="dram", bufs=1, space="DRAM") as dram,
        ):
            # ---------- load inputs ----------
            catT_sb = wpool.tile([P, KF * NB], BF16)
            bwT_sb = wpool.tile([P, KF * HS], BF16)
            wrowT_sb = wpool.tile([P, KH * HS], BF16)
            wcolN_sb = wpool.tile([P, KH * HS], BF16)
            cwT_sb = wpool.tile([P, KH * YS], F32R)
            v_sb = wpool.tile([P, MT], F32)
            nc.sync.dma_start(catT_sb[:], catT[:])
            nc.sync.dma_start(bwT_sb[:], bwT[:])
            nc.sync.dma_start(wrowT_sb[:], wrowT[:])
            nc.sync.dma_start(wcolN_sb[:], wcolN[:])
            nc.sync.dma_start(cwT_sb[:], cwT[:])
            nc.sync.dma_start(v_sb[:], vcol[:])

            # ---------- stage A: B0[I_c] (MT m-tiles) ----------
            # Z0 payload = [B0 | v] in bf16 for the AllGather
            z0_sb = []  # per m-tile (128, 513) bf16
            pA_list = []
            for mi in range(MT):
                pA = psA.tile([P, NB], F32, tag="psA")
                for kf in range(KF):
                    nc.tensor.matmul(
                        pA[:],
                        bwT_sb[:, kf * HS + mi * P : kf * HS + (mi + 1) * P],
                        catT_sb[:, kf * NB : (kf + 1) * NB],
                        start=(kf == 0),
                        stop=(kf == KF - 1),
                    )
                z0 = apool.tile([P, NB + 1], BF16, tag="z0", bufs=2)
                nc.vector.tensor_copy(z0[:, 0:NB], pA[:])
                nc.vector.tensor_copy(z0[:, NB : NB + 1], v_sb[:, mi : mi + 1])
                z0_sb.append(z0)
                pA_list.append(pA)  # keep alive: read again in combine

            # ---------- AllGather Z0 ----------
            ag0_in = dram.tile([HS, NB + 1], BF16)
            ag0_out = dram.tile([H_DIM, NB + 1], BF16)
            for mi in range(MT):
                nc.gpsimd.dma_start(ag0_in[mi * P : (mi + 1) * P, :], z0_sb[mi][:])
            nc.gpsimd.collective_compute(
                "AllGather",
                mybir.AluOpType.bypass,
                replica_groups=[list(range(N_CORES))],
                ins=[ag0_in.opt()],
                outs=[ag0_out.opt()],
            )
            # gathered -> SBUF, k-tile layout (128, KH, NB+1)
            z0g_sb = apool.tile([P, KH, NB + 1], BF16)
            ag0_view = ag0_out.rearrange("(k p) c -> p k c", p=P)
            nc.sync.dma_start(z0g_sb[:], ag0_view)

            # ---------- stage S1: Z1[I_c] = Abar @ Z0 ----------
            h1_sb = []
            for mi in range(MT):
                pM = psM.tile([P, NB], F32, tag="psM")
                pV = psV.tile([P, 1], F32, tag="psV")
                n_mm = 0
                for term_sb in (wrowT_sb, wcolN_sb):
                    for kf in range(KH):
                        lhsT = term_sb[:, kf * HS + mi * P : kf * HS + (mi + 1) * P]
                        rhs_blk = z0g_sb[:, kf]
                        first = n_mm == 0
                        last = n_mm == 2 * KH - 1
                        nc.tensor.matmul(
                            pM[:], lhsT, rhs_blk[:, 0:NB], start=first, stop=last
                        )
                        nc.tensor.matmul(
                            pV[:], lhsT, rhs_blk[:, NB : NB + 1], start=first, stop=last
                        )
                        n_mm += 1

                # combine: rec_col = v + (d/2) Z1v ; H1 = rec_col + d*B0 + (d^2/4) Z1
                cv = apool.tile([P, 1], F32, tag="cv", bufs=2)
                nc.scalar.activation(
                    cv[:],
                    pV[:],
                    mybir.ActivationFunctionType.Identity,
                    bias=v_sb[:, mi : mi + 1],
                    scale=d / 2.0,
                )
                t = apool.tile([P, NB], F32, tag="t", bufs=2)
                nc.scalar.activation(
                    t[:],
                    pM[:],
                    mybir.ActivationFunctionType.Identity,
                    bias=cv[:],
                    scale=d * d / 4.0,
                )
                h1 = apool.tile([P, NB], F32R, tag="h1", bufs=2)
                nc.vector.scalar_tensor_tensor(
                    h1[:],
                    pA_list[mi][:],
                    d,
                    t[:],
                    op0=mybir.AluOpType.mult,
                    op1=mybir.AluOpType.add,
                )
                h1_sb.append(h1)

            # ---------- AllGather H1 ----------
            ag1_in = dram.tile([HS, NB], F32R)
            ag1_out = dram.tile([H_DIM, NB], F32R)
            for mi in range(MT):
                nc.gpsimd.dma_start(ag1_in[mi * P : (mi + 1) * P, :], h1_sb[mi][:])
            nc.gpsimd.collective_compute(
                "AllGather",
                mybir.AluOpType.bypass,
                replica_groups=[list(range(N_CORES))],
                ins=[ag1_in.opt()],
                outs=[ag1_out.opt()],
            )
            h1g_sb = apool.tile([P, KH, NB], F32R)
            ag1_view = ag1_out.rearrange("(k p) c -> p k c", p=P)
            nc.sync.dma_start(h1g_sb[:], ag1_view)

            # ---------- stage C: yT[J_c] = Cw[J_c,:] @ H1 ----------
            pC = psC.tile([P, NB], F32)
            for kf in range(KH):
                nc.tensor.matmul(
                    pC[:],
                    cwT_sb[:, kf * YS : (kf + 1) * YS],
                    h1g_sb[:, kf],
                    start=(kf == 0),
                    stop=(kf == KH - 1),
                )
            y_sb = apool.tile([P, NB], F32)
            nc.vector.tensor_copy(y_sb[:], pC[:])
            nc.sync.dma_start(out[:], y_sb[:])

    nc.compile()
    return nc


_NC_CACHE = None


def _get_nc():
    global _NC_CACHE
    if _NC_CACHE is None:
        _NC_CACHE = build_nc()
    return _NC_CACHE


def kernel(u, du, W, Bw, Cw, h):
    u = np.asarray(u, dtype=np.float32)
    du = np.asarray(du, dtype=np.float32)
    W = np.asarray(W, dtype=np.float32)
    Bw = np.asarray(Bw, dtype=np.float32)
    Cw = np.asarray(Cw, dtype=np.float32)
    h = np.asarray(h, dtype=np.float32)

    cat = np.concatenate([du, u], axis=1)  # (B, F)
    catT = _to_sb_layout(np.ascontiguousarray(cat.T), BF)  # (F,B) -> sb layout

    in_maps = []
    for c in range(N_CORES):
        sl = slice(c * HS, (c + 1) * HS)
        bwT_c = _to_sb_layout(np.ascontiguousarray(Bw[sl, :].T), BF)
        wrowT_c = _to_sb_layout(np.ascontiguousarray(W[sl, :].T), BF)
        wcolN_c = _to_sb_layout(np.ascontiguousarray(-W[:, sl]), BF)
        ysl = slice(c * YS, (c + 1) * YS)
        cwT_c = _to_sb_layout(np.ascontiguousarray(Cw[ysl, :].T), np.float32)
        v_c = np.ascontiguousarray(
            h[0, sl].reshape(MT, P).T, dtype=np.float32
        )  # (128, MT)
        in_maps.append(
            {
                "catT": catT,
                "bwT": bwT_c,
                "wrowT": wrowT_c,
                "wcolN": wcolN_c,
                "cwT": cwT_c,
                "vcol": v_c,
            }
        )

    nc = _get_nc()
    res = run_bass_kernel_spmd(nc, in_maps, core_ids=list(range(N_CORES)))
    yT = np.concatenate([res.results[c]["out"] for c in range(N_CORES)], axis=0)
    return np.ascontiguousarray(yT.T)


# revision 13
# speedup vs baseline: 1.0054x; 1.0054x over previous
"""Trainium2 8-core Bass kernel for nn_AntisymmetricExpGenerator.

Reference computation (H=2048, B=512):
    A      = 0.5*(W - W.T)                      (antisymmetric)
    rec    = h @ expm(A*d).T
    b      = cat([du, u]) @ Bw.T
    M      = inv(A) @ (expm(A*d) - I)
    y      = (rec + b @ M.T) @ Cw.T

Key identity: inv(A) @ (expm(A*d) - I) = d * phi1(A*d) where
phi1(z) = (e^z - 1)/z = sum_k z^k/(k+1)!  is ENTIRE - no inverse and no
dense (H,H) expm/inv is needed.  With ||A*d|| ~ 8e-3 the series
converges after 2-3 terms (truncation ~1e-5 relative, far below fp32
matmul noise):

    b @ M.T = d*(b + (d/4)*b@Abar.T + O(1e-5))        Abar = W - W.T
    rec     = h + (d/2)*h@Abar.T + O(3e-5)

so everything reduces to skinny matmuls of the batch block against
Abar - never a 2048^3 product.

Distribution (8 cores): H dim sharded 256 rows/core.  Activations live
transposed (feature on partitions, batch on free dim).  Two AllGathers
(the B0|h block, then the H1 block) move (H,513)/(H,512) blocks;
weights are pre-sliced per core on the host (layout only).

dtypes: suppressed paths (scaled by d=0.01) run bf16; the direct path
(H1 -> y through Cw) runs float32r (TF32-like, ~1.5e-4) with fp32 PSUM
accumulation.  End-to-end error vs reference ~2e-4.
"""

import sys

sys.path.insert(0, "/opt/trn_rl_repo")

import numpy as np
import ml_dtypes

import concourse.bass as bass
import concourse.mybir as mybir
import concourse.tile as tile
from concourse import bacc
from concourse.bass_utils import run_bass_kernel_spmd

# problem constants (hardcoded per harness contract)
DELTA = 0.01
B_SZ, U_DIM, DU_DIM, H_DIM, Y_DIM = 512, 1024, 512, 2048, 1024
F_DIM = U_DIM + DU_DIM  # 1536
N_CORES = 8
HS = H_DIM // N_CORES  # 256 rows of H per core
YS = Y_DIM // N_CORES  # 128 rows of y^T per core

F32 = mybir.dt.float32
F32R = mybir.dt.float32r
BF16 = mybir.dt.bfloat16
BF = ml_dtypes.bfloat16

P = 128
NB = B_SZ  # batch free-dim (=512, one PSUM bank of fp32)
KF = F_DIM // P  # 12 k-tiles for stage A
KH = H_DIM // P  # 16 k-tiles for H-contractions
MT = HS // P  # 2 m-tiles per core for H-sharded outputs


def _to_sb_layout(a: np.ndarray, dtype) -> np.ndarray:
    """(K, M) -> (128, (K//128)*M): k-tile kf lands at cols [kf*M,(kf+1)*M).

    Matches an SBUF tile (128, K//128 * M) where slice [:, kf*M:(kf+1)*M]
    is the (128, M) k-tile needed by the tensor engine.
    """
    K, M = a.shape
    assert K % P == 0
    return np.ascontiguousarray(
        a.reshape(K // P, P, M).transpose(1, 0, 2).reshape(P, (K // P) * M)
    ).astype(dtype, copy=False)


def build_nc():
    nc = bacc.Bacc("TRN2", target_bir_lowering=False, debug=False, num_devices=N_CORES)

    # --- per-core DRAM parameters (host-prepared layouts) ---
    # stage A: B0[I_c] = Bw[I_c,:] @ cat^T ;  lhsT = Bw[I_c,:].T  (F, HS)
    catT = nc.dram_tensor("catT", [P, KF * NB], BF16, kind="ExternalInput")
    bwT = nc.dram_tensor("bwT", [P, KF * HS], BF16, kind="ExternalInput")
    # stage S1: Z1[I_c] = W[I_c,:] @ Z0 - W[:,I_c].T @ Z0
    #   lhsT(term1) = (W[I_c,:]).T  (H, HS);  lhsT(term2) = -W[:,I_c]  (H, HS)
    wrowT = nc.dram_tensor("wrowT", [P, KH * HS], BF16, kind="ExternalInput")
    wcolN = nc.dram_tensor("wcolN", [P, KH * HS], BF16, kind="ExternalInput")
    # stage C: yT[J_c] = Cw[J_c,:] @ H1 ; lhsT = Cw[J_c,:].T  (H, YS)
    cwT = nc.dram_tensor("cwT", [P, KH * YS], F32R, kind="ExternalInput")
    # h vector slice for this core, (HS,1) -> stored (128, MT)
    vcol = nc.dram_tensor("vcol", [P, MT], F32, kind="ExternalInput")

    out = nc.dram_tensor("out", [YS, NB], F32, kind="ExternalOutput")

    d = DELTA

    with tile.TileContext(nc) as tc:
        with (
            tc.tile_pool(name="weights", bufs=1) as wpool,
            tc.tile_pool(name="acts", bufs=1) as apool,
            tc.tile_pool(name="psumA", bufs=2, space="PSUM") as psA,
            tc.tile_pool(name="psumM", bufs=2, space="PSUM") as psM,
            tc.tile_pool(name="psumV", bufs=2, space="PSUM") as psV,
            tc.tile_pool(name="psumC", bufs=1, space="PSUM") as psC,
            tc.tile_pool(name="dram", bufs=1, space="DRAM") as dram,
        ):
            # ---------- warm up the collectives engine ----------
            # The first collective of a NEFF execution pays a ~30us global
            # entry barrier on the CC cores.  Fire a tiny dummy AllGather
            # immediately so the barrier overlaps input DMA + stage A
            # instead of gating the first real AllGather.
            warm_in = dram.tile([1, 32], F32)
            warm_out = dram.tile([N_CORES, 32], F32, addr_space="Shared")
            warm_src = wpool.tile([1, 32], F32)
            nc.gpsimd.memset(warm_src[:], 0.0)
            nc.gpsimd.dma_start(warm_in[:], warm_src[:])
            nc.gpsimd.collective_compute(
                "AllGather",
                mybir.AluOpType.bypass,
                replica_groups=[list(range(N_CORES))],
                ins=[warm_in.opt()],
                outs=[warm_out.opt()],
            )

            # ---------- load inputs (per k-tile so compute starts early) ----
            catT_sb = [apool.tile([P, NB], BF16, tag="catT", bufs=KF, name=f"catT_sb{i}") for i in range(KF)]
            bwT_sb = [apool.tile([P, HS], BF16, tag="bwT", bufs=KF, name=f"bwT_sb{i}") for i in range(KF)]
            v_sb = wpool.tile([P, MT], F32)
            nc.sync.dma_start(v_sb[:], vcol[:])
            for kf in range(KF):
                nc.sync.dma_start(catT_sb[kf][:], catT[:, kf * NB : (kf + 1) * NB])
                nc.sync.dma_start(bwT_sb[kf][:], bwT[:, kf * HS : (kf + 1) * HS])
            wrowT_sb = [apool.tile([P, HS], BF16, tag="wrowT", bufs=KH, name=f"wrowT_sb{i}") for i in range(KH)]
            wcolN_sb = [apool.tile([P, HS], BF16, tag="wcolN", bufs=KH, name=f"wcolN_sb{i}") for i in range(KH)]
            cwT_sb = [apool.tile([P, YS], F32R, tag="cwT", bufs=KH, name=f"cwT_sb{i}") for i in range(KH)]
            for kf in range(KH):
                nc.sync.dma_start(wrowT_sb[kf][:], wrowT[:, kf * HS : (kf + 1) * HS])
                nc.sync.dma_start(wcolN_sb[kf][:], wcolN[:, kf * HS : (kf + 1) * HS])
                nc.sync.dma_start(cwT_sb[kf][:], cwT[:, kf * YS : (kf + 1) * YS])

            # ---------- stage A: B0[I_c] (MT m-tiles) ----------
            # Z0 payload = [B0 | v] in bf16 for the AllGather
            z0_sb = []  # per m-tile (128, 513) bf16
            pA_list = []
            for mi in range(MT):
                pA = psA.tile([P, NB], F32, tag="psA")
                for kf in range(KF):
                    nc.tensor.matmul(
                        pA[:],
                        bwT_sb[kf][:, mi * P : (mi + 1) * P],
                        catT_sb[kf][:],
                        start=(kf == 0),
                        stop=(kf == KF - 1),
                    )
                z0 = apool.tile([P, NB + 1], BF16, tag="z0", bufs=2)
                nc.vector.tensor_copy(z0[:, 0:NB], pA[:])
                nc.vector.tensor_copy(z0[:, NB : NB + 1], v_sb[:, mi : mi + 1])
                z0_sb.append(z0)
                pA_list.append(pA)  # keep alive: read again in combine

            # ---------- AllGather Z0 ----------
            ag0_in = dram.tile([HS, NB + 1], BF16)
            ag0_out = dram.tile([H_DIM, NB + 1], BF16, addr_space="Shared")
            for mi in range(MT):
                nc.gpsimd.dma_start(ag0_in[mi * P : (mi + 1) * P, :], z0_sb[mi][:])
            nc.gpsimd.collective_compute(
                "AllGather",
                mybir.AluOpType.bypass,
                replica_groups=[list(range(N_CORES))],
                ins=[ag0_in.opt()],
                outs=[ag0_out.opt()],
            )
            # gathered -> SBUF, k-tile layout (128, KH, NB+1)
            z0g_sb = apool.tile([P, KH, NB + 1], BF16)
            ag0_view = ag0_out.rearrange("(k p) c -> p k c", p=P)
            nc.sync.dma_start(z0g_sb[:], ag0_view)

            # ---------- stage S1: Z1[I_c] = Abar @ Z0 ----------
            h1_sb = []
            for mi in range(MT):
                pM = psM.tile([P, NB], F32, tag="psM")
                pV = psV.tile([P, 1], F32, tag="psV")
                n_mm = 0
                for term_sb in (wrowT_sb, wcolN_sb):
                    for kf in range(KH):
                        lhsT = term_sb[kf][:, mi * P : (mi + 1) * P]
                        rhs_blk = z0g_sb[:, kf]
                        first = n_mm == 0
                        last = n_mm == 2 * KH - 1
                        nc.tensor.matmul(
                            pM[:], lhsT, rhs_blk[:, 0:NB], start=first, stop=last
                        )
                        nc.tensor.matmul(
                            pV[:], lhsT, rhs_blk[:, NB : NB + 1], start=first, stop=last
                        )
                        n_mm += 1

                # combine: rec_col = v + (d/2) Z1v ; H1 = rec_col + d*B0 + (d^2/4) Z1
                cv = apool.tile([P, 1], F32, tag="cv", bufs=2)
                nc.scalar.activation(
                    cv[:],
                    pV[:],
                    mybir.ActivationFunctionType.Identity,
                    bias=v_sb[:, mi : mi + 1],
                    scale=d / 2.0,
                )
                t = apool.tile([P, NB], F32, tag="t", bufs=2)
                nc.scalar.activation(
                    t[:],
                    pM[:],
                    mybir.ActivationFunctionType.Identity,
                    bias=cv[:],
                    scale=d * d / 4.0,
                )
                h1 = apool.tile([P, NB], F32R, tag="h1", bufs=2)
                nc.vector.scalar_tensor_tensor(
                    h1[:],
                    pA_list[mi][:],
                    d,
                    t[:],
                    op0=mybir.AluOpType.mult,
                    op1=mybir.AluOpType.add,
                )
                h1_sb.append(h1)

            # ---------- AllGather H1 ----------
            ag1_in = dram.tile([HS, NB], F32R)
            ag1_out = dram.tile([H_DIM, NB], F32R, addr_space="Shared")
            for mi in range(MT):
                nc.gpsimd.dma_start(ag1_in[mi * P : (mi + 1) * P, :], h1_sb[mi][:])
            nc.gpsimd.collective_compute(
                "AllGather",
                mybir.AluOpType.bypass,
                replica_groups=[list(range(N_CORES))],
                ins=[ag1_in.opt()],
                outs=[ag1_out.opt()],
            )
            h1g_sb = apool.tile([P, KH, NB], F32R)
            ag1_view = ag1_out.rearrange("(k p) c -> p k c", p=P)
            nc.sync.dma_start(h1g_sb[:], ag1_view)

            # ---------- stage C: yT[J_c] = Cw[J_c,:] @ H1 ----------
            pC = psC.tile([P, NB], F32)
            for kf in range(KH):
                nc.tensor.matmul(
                    pC[:],
                    cwT_sb[kf][:],
                    h1g_sb[:, kf],
                    start=(kf == 0),
                    stop=(kf == KH - 1),
                )
            y_sb = apool.tile([P, NB], F32)
            nc.vector.tensor_copy(y_sb[:], pC[:])
            nc.sync.dma_start(out[:], y_sb[:])

    nc.compile()
    return nc


_NC_CACHE = None


def _get_nc():
    global _NC_CACHE
    if _NC_CACHE is None:
        _NC_CACHE = build_nc()
    return _NC_CACHE


def kernel(u, du, W, Bw, Cw, h):
    u = np.asarray(u, dtype=np.float32)
    du = np.asarray(du, dtype=np.float32)
    W = np.asarray(W, dtype=np.float32)
    Bw = np.asarray(Bw, dtype=np.float32)
    Cw = np.asarray(Cw, dtype=np.float32)
    h = np.asarray(h, dtype=np.float32)

    cat = np.concatenate([du, u], axis=1)  # (B, F)
    catT = _to_sb_layout(np.ascontiguousarray(cat.T), BF)  # (F,B) -> sb layout

    in_maps = []
    for c in range(N_CORES):
        sl = slice(c * HS, (c + 1) * HS)
        bwT_c = _to_sb_layout(np.ascontiguousarray(Bw[sl, :].T), BF)
        wrowT_c = _to_sb_layout(np.ascontiguousarray(W[sl, :].T), BF)
        wcolN_c = _to_sb_layout(np.ascontiguousarray(-W[:, sl]), BF)
        ysl = slice(c * YS, (c + 1) * YS)
        cwT_c = _to_sb_layout(np.ascontiguousarray(Cw[ysl, :].T), np.float32)
        v_c = np.ascontiguousarray(
            h[0, sl].reshape(MT, P).T, dtype=np.float32
        )  # (128, MT)
        in_maps.append(
            {
                "catT": catT,
                "bwT": bwT_c,
                "wrowT": wrowT_c,
                "wcolN": wcolN_c,
                "cwT": cwT_c,
                "vcol": v_c,
            }
        )

    nc = _get_nc()
    res = run_bass_kernel_spmd(nc, in_maps, core_ids=list(range(N_CORES)))
    yT = np.concatenate([res.results[c]["out"] for c in range(N_CORES)], axis=0)
    return np.ascontiguousarray(yT.T)


# revision 15
# speedup vs baseline: 1.1005x; 1.0947x over previous
"""Trainium2 8-core Bass kernel for nn_AntisymmetricExpGenerator.

Reference computation (H=2048, B=512):
    A      = 0.5*(W - W.T)                      (antisymmetric)
    rec    = h @ expm(A*d).T
    b      = cat([du, u]) @ Bw.T
    M      = inv(A) @ (expm(A*d) - I)
    y      = (rec + b @ M.T) @ Cw.T

Key identity: inv(A) @ (expm(A*d) - I) = d * phi1(A*d) where
phi1(z) = (e^z - 1)/z = sum_k z^k/(k+1)!  is ENTIRE - no inverse and no
dense (H,H) expm/inv is needed.  With ||A*d|| ~ 8e-3 the series
converges after 2 terms (truncation ~1e-5 relative, far below the fp32
matmul noise of the reference itself):

    b @ M.T = d*(b + (d/4)*b@Abar.T + O(1e-5))        Abar = W - W.T
    rec     = h + (d/2)*h@Abar.T + O(3e-5)

so everything reduces to skinny matmuls of the batch block against
Abar - never a 2048^3 product.

Distribution (8 cores): H dim sharded 256 rows/core.  Activations live
transposed (feature on partitions, batch on free dim).  Weights are
pre-sliced per core on the host (layout only).  Two AllGather stages
(the [B0|h] block, then the H1 block) are each split into two
batch-column halves so collectives pipeline against the S1/C matmuls;
the first collective additionally hides part of the runtime's CC entry
barrier.

The h-vector chain rides as PSUM column 256 of the half-A matmuls
(N=257), so no separate mat-vec work exists.  The H1 AllGather ships
bf16 `inp = H1 - rec` (|inp| ~ 0.006 so bf16 error is suppressed 170x)
plus an exact hi/lo bf16 split of the f32 rec column; H1 is
reconstructed to float32r on-device.  Direct-path matmuls (Cw @ H1) run
float32r (TF32-like); end-to-end error vs reference ~2e-4.
"""

import sys

sys.path.insert(0, "/opt/trn_rl_repo")

import numpy as np
import ml_dtypes

import concourse.bass as bass
import concourse.mybir as mybir
import concourse.tile as tile
from concourse import bacc
from concourse.bass_utils import run_bass_kernel_spmd

# problem constants (hardcoded per harness contract)
DELTA = 0.01
B_SZ, U_DIM, DU_DIM, H_DIM, Y_DIM = 512, 1024, 512, 2048, 1024
F_DIM = U_DIM + DU_DIM  # 1536
N_CORES = 8
HS = H_DIM // N_CORES  # 256 rows of H per core
YS = Y_DIM // N_CORES  # 128 rows of y^T per core

F32 = mybir.dt.float32
F32R = mybir.dt.float32r
BF16 = mybir.dt.bfloat16
BF = ml_dtypes.bfloat16

P = 128
NB = B_SZ  # batch free dim (512)
NBH = NB // 2  # batch half (256)
KF = F_DIM // P  # 12 k-tiles for stage A
KH = H_DIM // P  # 16 k-tiles for H-contractions
MT = HS // P  # 2 m-tiles per core for H-sharded outputs
RG = [list(range(N_CORES))]


def _to_sb_layout(a: np.ndarray, dtype) -> np.ndarray:
    """(K, M) -> (128, (K//128)*M): k-tile kf lands at cols [kf*M,(kf+1)*M)."""
    K, M = a.shape
    assert K % P == 0
    return np.ascontiguousarray(
        a.reshape(K // P, P, M).transpose(1, 0, 2).reshape(P, (K // P) * M)
    ).astype(dtype, copy=False)


def build_nc():
    nc = bacc.Bacc("TRN2", target_bir_lowering=False, debug=False, num_devices=N_CORES)

    # --- per-core DRAM parameters (host-prepared layouts) ---
    catT = nc.dram_tensor("catT", [P, KF * NB], BF16, kind="ExternalInput")
    bwT = nc.dram_tensor("bwT", [P, KF * HS], BF16, kind="ExternalInput")
    wrowT = nc.dram_tensor("wrowT", [P, KH * HS], BF16, kind="ExternalInput")
    wcolN = nc.dram_tensor("wcolN", [P, KH * HS], BF16, kind="ExternalInput")
    cwT = nc.dram_tensor("cwT", [P, KH * YS], F32R, kind="ExternalInput")
    vcol = nc.dram_tensor("vcol", [P, MT], F32, kind="ExternalInput")

    out = nc.dram_tensor("out", [YS, NB], F32, kind="ExternalOutput")

    d = DELTA

    with tile.TileContext(nc) as tc:
        with (
            tc.tile_pool(name="wpool", bufs=1) as wpool,
            tc.tile_pool(name="acts", bufs=1) as apool,
            tc.tile_pool(name="psumA", bufs=2, space="PSUM") as psA,
            tc.tile_pool(name="psumM", bufs=4, space="PSUM") as psM,
            tc.tile_pool(name="psumC", bufs=2, space="PSUM") as psC,
            tc.tile_pool(name="dram", bufs=1, space="DRAM") as dram,
        ):
            # ---------- load inputs (per k-tile so compute starts early) ----
            catT_sb = [
                apool.tile([P, NB], BF16, tag="catT", bufs=KF, name=f"catT_sb{i}")
                for i in range(KF)
            ]
            bwT_sb = [
                apool.tile([P, HS], BF16, tag="bwT", bufs=KF, name=f"bwT_sb{i}")
                for i in range(KF)
            ]
            v_sb = wpool.tile([P, MT], F32)
            nc.sync.dma_start(v_sb[:], vcol[:])
            for kf in range(KF):
                nc.sync.dma_start(catT_sb[kf][:], catT[:, kf * NB : (kf + 1) * NB])
                nc.sync.dma_start(bwT_sb[kf][:], bwT[:, kf * HS : (kf + 1) * HS])
            wrowT_sb = [
                apool.tile([P, HS], BF16, tag="wrowT", bufs=KH, name=f"wrowT_sb{i}")
                for i in range(KH)
            ]
            wcolN_sb = [
                apool.tile([P, HS], BF16, tag="wcolN", bufs=KH, name=f"wcolN_sb{i}")
                for i in range(KH)
            ]
            cwT_sb = [
                apool.tile([P, YS], F32R, tag="cwT", bufs=KH, name=f"cwT_sb{i}")
                for i in range(KH)
            ]
            for kf in range(KH):
                nc.sync.dma_start(wrowT_sb[kf][:], wrowT[:, kf * HS : (kf + 1) * HS])
                nc.sync.dma_start(wcolN_sb[kf][:], wcolN[:, kf * HS : (kf + 1) * HS])
                nc.sync.dma_start(cwT_sb[kf][:], cwT[:, kf * YS : (kf + 1) * YS])

            # ---------- stage A: B0[I_c] ----------
            pA_list = []
            z0a_sb = []  # (128, 257): B0 cols 0:256 + v col
            z0b_sb = []  # (128, 256): B0 cols 256:512
            for mi in range(MT):
                pA = psA.tile([P, NB], F32, tag="psA", name=f"pA{mi}")
                for kf in range(KF):
                    nc.tensor.matmul(
                        pA[:],
                        bwT_sb[kf][:, mi * P : (mi + 1) * P],
                        catT_sb[kf][:],
                        start=(kf == 0),
                        stop=(kf == KF - 1),
                    )
                z0a = apool.tile([P, NBH + 1], BF16, tag="z0a", bufs=2, name=f"z0a{mi}")
                z0b = apool.tile([P, NBH], BF16, tag="z0b", bufs=2, name=f"z0b{mi}")
                nc.vector.tensor_copy(z0a[:, 0:NBH], pA[:, 0:NBH])
                nc.vector.tensor_copy(z0a[:, NBH : NBH + 1], v_sb[:, mi : mi + 1])
                nc.vector.tensor_copy(z0b[:], pA[:, NBH:NB])
                pA_list.append(pA)
                z0a_sb.append(z0a)
                z0b_sb.append(z0b)

            # ---------- AllGather Z0 (two batch halves) ----------
            ag0a_in = dram.tile([HS, NBH + 1], BF16)
            ag0a_out = dram.tile([H_DIM, NBH + 1], BF16, addr_space="Shared")
            ag0b_in = dram.tile([HS, NBH], BF16)
            ag0b_out = dram.tile([H_DIM, NBH], BF16, addr_space="Shared")
            for mi in range(MT):
                nc.gpsimd.dma_start(ag0a_in[mi * P : (mi + 1) * P, :], z0a_sb[mi][:])
            nc.gpsimd.collective_compute(
                "AllGather", mybir.AluOpType.bypass, replica_groups=RG,
                ins=[ag0a_in.opt()], outs=[ag0a_out.opt()],
            )
            for mi in range(MT):
                nc.gpsimd.dma_start(ag0b_in[mi * P : (mi + 1) * P, :], z0b_sb[mi][:])
            nc.gpsimd.collective_compute(
                "AllGather", mybir.AluOpType.bypass, replica_groups=RG,
                ins=[ag0b_in.opt()], outs=[ag0b_out.opt()],
            )
            # gathered -> SBUF per k-tile (dep granularity for pipelining)
            z0ga_sb = [
                apool.tile([P, NBH + 1], BF16, tag="z0ga", bufs=KH, name=f"z0ga{i}")
                for i in range(KH)
            ]
            z0gb_sb = [
                apool.tile([P, NBH], BF16, tag="z0gb", bufs=KH, name=f"z0gb{i}")
                for i in range(KH)
            ]
            for kf in range(KH):
                nc.sync.dma_start(z0ga_sb[kf][:], ag0a_out[kf * P : (kf + 1) * P, :])
                nc.sync.dma_start(z0gb_sb[kf][:], ag0b_out[kf * P : (kf + 1) * P, :])

            # ---------- stage S1: Z1[I_c] = Abar @ Z0, half A then half B ----
            pMa = []
            pMb = []
            for mi in range(MT):
                pMa.append(psM.tile([P, NBH + 1], F32, tag="psM", name=f"pMa{mi}"))
                pMb.append(psM.tile([P, NBH], F32, tag="psM", name=f"pMb{mi}"))
            for half in range(2):
                for mi in range(MT):
                    pM = (pMa if half == 0 else pMb)[mi]
                    zg = z0ga_sb if half == 0 else z0gb_sb
                    n_mm = 0
                    for term_sb in (wrowT_sb, wcolN_sb):
                        for kf in range(KH):
                            nc.tensor.matmul(
                                pM[:],
                                term_sb[kf][:, mi * P : (mi + 1) * P],
                                zg[kf][:],
                                start=(n_mm == 0),
                                stop=(n_mm == 2 * KH - 1),
                            )
                            n_mm += 1

            # ---------- combine ----------
            # rec_col = v + (d/2) Z1v  (exact f32, shipped as bf16 hi+lo)
            # inp     = d*B0 + (d^2/4) Z1   (bf16: |inp|~0.006, error suppressed)
            z1a_pay = []
            z1b_pay = []
            for mi in range(MT):
                cv = apool.tile([P, 1], F32, tag="cv", bufs=MT, name=f"cv{mi}")
                nc.scalar.activation(
                    cv[:],
                    pMa[mi][:, NBH : NBH + 1],
                    mybir.ActivationFunctionType.Identity,
                    bias=v_sb[:, mi : mi + 1],
                    scale=d / 2.0,
                )
                paya = apool.tile(
                    [P, NBH + 2], BF16, tag="paya", bufs=MT, name=f"paya{mi}"
                )
                payb = apool.tile([P, NBH], BF16, tag="payb", bufs=MT, name=f"payb{mi}")
                # hi/lo split of cv into payload cols 256/257
                hi_f = apool.tile([P, 1], F32, tag="hi_f", bufs=MT, name=f"hi_f{mi}")
                nc.vector.tensor_copy(paya[:, NBH : NBH + 1], cv[:])  # f32->bf16 round
                nc.vector.tensor_copy(hi_f[:], paya[:, NBH : NBH + 1])  # back to f32
                nc.vector.tensor_sub(paya[:, NBH + 1 : NBH + 2], cv[:], hi_f[:])
                for half in range(2):
                    pM = (pMa if half == 0 else pMb)[mi]
                    pay = paya if half == 0 else payb
                    t = apool.tile(
                        [P, NBH], F32, tag="t", bufs=2 * MT, name=f"t{mi}_{half}"
                    )
                    nc.scalar.activation(
                        t[:],
                        pM[:, 0:NBH],
                        mybir.ActivationFunctionType.Identity,
                        bias=0.0,
                        scale=d * d / 4.0,
                    )
                    nc.vector.scalar_tensor_tensor(
                        pay[:, 0:NBH],
                        pA_list[mi][:, half * NBH : (half + 1) * NBH],
                        d,
                        t[:],
                        op0=mybir.AluOpType.mult,
                        op1=mybir.AluOpType.add,
                    )
                z1a_pay.append(paya)
                z1b_pay.append(payb)

            # ---------- AllGather H1 (inp + rec cols), two halves ----------
            ag1a_in = dram.tile([HS, NBH + 2], BF16)
            ag1a_out = dram.tile([H_DIM, NBH + 2], BF16, addr_space="Shared")
            ag1b_in = dram.tile([HS, NBH], BF16)
            ag1b_out = dram.tile([H_DIM, NBH], BF16, addr_space="Shared")
            for mi in range(MT):
                nc.gpsimd.dma_start(ag1a_in[mi * P : (mi + 1) * P, :], z1a_pay[mi][:])
            nc.gpsimd.collective_compute(
                "AllGather", mybir.AluOpType.bypass, replica_groups=RG,
                ins=[ag1a_in.opt()], outs=[ag1a_out.opt()],
            )
            for mi in range(MT):
                nc.gpsimd.dma_start(ag1b_in[mi * P : (mi + 1) * P, :], z1b_pay[mi][:])
            nc.gpsimd.collective_compute(
                "AllGather", mybir.AluOpType.bypass, replica_groups=RG,
                ins=[ag1b_in.opt()], outs=[ag1b_out.opt()],
            )

            # ---------- stage C: yT[J_c] = Cw[J_c,:] @ H1 ----------
            y_sb = apool.tile([P, NB], F32, tag="y", name="y_sb")
            rec_cols = []
            for half in range(2):
                ag_out = ag1a_out if half == 0 else ag1b_out
                w = NBH + 2 if half == 0 else NBH
                pC = psC.tile([P, NBH], F32, tag="psC", name=f"pC{half}")
                for kf in range(KH):
                    g = apool.tile(
                        [P, w], BF16, tag="g", bufs=4, name=f"g{half}_{kf}"
                    )
                    nc.sync.dma_start(g[:], ag_out[kf * P : (kf + 1) * P, :])
                    if half == 0:
                        rec_col = apool.tile(
                            [P, 1], F32, tag="rec", bufs=KH, name=f"rec{kf}"
                        )
                        nc.vector.tensor_add(
                            rec_col[:], g[:, NBH : NBH + 1], g[:, NBH + 1 : NBH + 2]
                        )
                        rec_cols.append(rec_col)
                    h1 = apool.tile(
                        [P, NBH], F32R, tag="h1", bufs=4, name=f"h1_{half}_{kf}"
                    )
                    nc.vector.tensor_scalar(
                        h1[:],
                        g[:, 0:NBH],
                        rec_cols[kf][:],
                        None,
                        op0=mybir.AluOpType.add,
                    )
                    nc.tensor.matmul(
                        pC[:],
                        cwT_sb[kf][:],
                        h1[:],
                        start=(kf == 0),
                        stop=(kf == KH - 1),
                    )
                nc.vector.tensor_copy(y_sb[:, half * NBH : (half + 1) * NBH], pC[:])
            nc.sync.dma_start(out[:], y_sb[:])

    nc.compile()
    return nc


_NC_CACHE = None


def _get_nc():
    global _NC_CACHE
    if _NC_CACHE is None:
        _NC_CACHE = build_nc()
    return _NC_CACHE


def make_in_maps(u, du, W, Bw, Cw, h):
    cat = np.concatenate([du, u], axis=1)  # (B, F)
    catT = _to_sb_layout(np.ascontiguousarray(cat.T), BF)
    in_maps = []
    for c in range(N_CORES):
        sl = slice(c * HS, (c + 1) * HS)
        ysl = slice(c * YS, (c + 1) * YS)
        in_maps.append(
            {
                "catT": catT,
                "bwT": _to_sb_layout(np.ascontiguousarray(Bw[sl, :].T), BF),
                "wrowT": _to_sb_layout(np.ascontiguousarray(W[sl, :].T), BF),
                "wcolN": _to_sb_layout(np.ascontiguousarray(-W[:, sl]), BF),
                "cwT": _to_sb_layout(np.ascontiguousarray(Cw[ysl, :].T), np.float32),
                "vcol": np.ascontiguousarray(
                    h[0, sl].reshape(MT, P).T, dtype=np.float32
                ),
            }
        )
    return in_maps


def kernel(u, du, W, Bw, Cw, h):
    u = np.asarray(u, dtype=np.float32)
    du = np.asarray(du, dtype=np.float32)
    W = np.asarray(W, dtype=np.float32)
    Bw = np.asarray(Bw, dtype=np.float32)
    Cw = np.asarray(Cw, dtype=np.float32)
    h = np.asarray(h, dtype=np.float32)

    in_maps = make_in_maps(u, du, W, Bw, Cw, h)
    nc = _get_nc()
    res = run_bass_kernel_spmd(nc, in_maps, core_ids=list(range(N_CORES)))
    yT = np.concatenate([res.results[c]["out"] for c in range(N_CORES)], axis=0)
    return np.ascontiguousarray(yT.T)
